# revision 3
# baseline (speedup 1.0000x reference)
"""Kalman-filter video model on 8 Trainium2 NeuronCores.

Strategy: data/model-parallel execution via PJRT on the 8 axon NeuronCores.
The frame-parallel parts (transformer uncertainty estimator + gain CNN) are
sharded over the (b f) frame-batch axis across the 8 cores per the sharding
hint; GSPMD inserts the all-to-all for the temporal attention and the
per-frame syncs for the sequential predict/update scan, which is replicated
(B=1) since it is latency-bound.
"""
import numpy as np

B, F, C, H, W = 1, 8, 256, 32, 32
HEADS, HEAD_DIM, LAYERS = 4, 64, 4
DIM = C
INNER = HEADS * HEAD_DIM
FF = 4 * DIM

_COMPILED = {}


def _build():
    import jax
    import jax.numpy as jnp
    from jax import lax
    from jax.sharding import Mesh, PartitionSpec as P, NamedSharding

    def layer_norm(x, p, eps=1e-5):
        m = jnp.mean(x, -1, keepdims=True)
        v = jnp.var(x, -1, keepdims=True)
        return (x - m) * lax.rsqrt(v + eps) * p["g"] + p["b"]

    def attn_core(q, k, v, heads):
        b, sq, _ = q.shape
        split = lambda t: t.reshape(t.shape[0], t.shape[1], heads, -1).transpose(0, 2, 1, 3)
        qh, kh, vh = split(q), split(k), split(v)
        scale = qh.shape[-1] ** -0.5
        a = jax.nn.softmax(jnp.einsum('bhqd,bhkd->bhqk', qh, kh) * scale, axis=-1)
        o = jnp.einsum('bhqk,bhkd->bhqd', a, vh)
        return o.transpose(0, 2, 1, 3).reshape(b, sq, -1)

    def sparse_causal_attn(x, p, f):
        bf, d, c = x.shape
        b = bf // f
        q = x @ p["wq"]
        k = (x @ p["wk"]).reshape(b, f, d, -1)
        v = (x @ p["wv"]).reshape(b, f, d, -1)
        idx0 = jnp.zeros((f,), jnp.int32)
        idxp = jnp.clip(jnp.arange(f) - 1, 0, f - 1)
        k = jnp.concatenate([k[:, idx0], k[:, idxp]], axis=2).reshape(bf, 2 * d, -1)
        v = jnp.concatenate([v[:, idx0], v[:, idxp]], axis=2).reshape(bf, 2 * d, -1)
        return attn_core(q, k, v, HEADS) @ p["wo"] + p["bo"]

    def self_attn(x, p):
        o = attn_core(x @ p["wq"], x @ p["wk"], x @ p["wv"], HEADS)
        return o @ p["wo"] + p["bo"]

    def geglu_ff(x, p):
        h = x @ p["w1"] + p["b1"]
        a, g = jnp.split(h, 2, axis=-1)
        return (a * jax.nn.gelu(g)) @ p["w2"] + p["b2"]

    def transformer_block(x, p, f):
        h = x + sparse_causal_attn(layer_norm(x, p["norm1"]), p["attn1"], f)
        h = h + geglu_ff(layer_norm(h, p["norm3"]), p["ff"])
        bf, d, c = h.shape
        b = bf // f
        ht = h.reshape(b, f, d, c).transpose(0, 2, 1, 3).reshape(b * d, f, c)
        ht = ht + self_attn(layer_norm(ht, p["norm_temp"]), p["attn_temp"])
        return ht.reshape(b, d, f, c).transpose(0, 2, 1, 3).reshape(bf, d, c)

    def group_norm(x, p, groups, eps=1e-6):
        b, c, h, w = x.shape
        xr = x.reshape(b, groups, -1)
        m = jnp.mean(xr, -1, keepdims=True)
        v = jnp.var(xr, -1, keepdims=True)
        xn = ((xr - m) * lax.rsqrt(v + eps)).reshape(b, c, h, w)
        return xn * p["g"][None, :, None, None] + p["b"][None, :, None, None]

    def conv2d(x, p, pad):
        y = lax.conv_general_dilated(x, p["w"], (1, 1), [(pad, pad), (pad, pad)],
                                     dimension_numbers=('NCHW', 'OIHW', 'NCHW'))
        return y + p["b"][None, :, None, None]

    def res_block(x, p):
        groups = C // 4
        h = conv2d(jax.nn.silu(group_norm(x, p["n1"], groups)), p["c1"], 1)
        h = conv2d(jax.nn.silu(group_norm(h, p["n2"], groups)), p["c2"], 1)
        return x + h

    def calc_gain(z_codes, params):
        b, f, c, h, w = z_codes.shape
        x = z_codes.reshape(b * f, c, h * w).transpose(0, 2, 1)
        for blk in params["blocks"]:
            x = transformer_block(x, blk, f)
        x = x.transpose(0, 2, 1).reshape(b * f, c, h, w)
        for rp in params["gain_res"]:
            x = res_block(x, rp)
        gain = jax.nn.sigmoid(conv2d(x, params["gain_out"], 0))
        return gain.reshape(b, f, 1, h, w)

    def predictor(z, params):
        for rp in params["pred_res"]:
            z = res_block(z, rp)
        return jax.nn.sigmoid(z)

    def model(z_codes, params):
        gains = calc_gain(z_codes, params)
        z0 = z_codes[:, 0]

        def step(z_hat, inp):
            z_t, k_t = inp
            z_prime = predictor(z_hat, params)
            z_new = (1.0 - k_t) * z_t + k_t * z_prime
            return z_new, z_new

        xs = (z_codes[:, 1:].transpose(1, 0, 2, 3, 4), gains[:, 1:].transpose(1, 0, 2, 3, 4))
        _, ys = lax.scan(step, z0, xs)
        out = jnp.concatenate([z0[None], ys], axis=0).transpose(1, 0, 2, 3, 4)
        return out

    devs = jax.devices()[:8]
    mesh = Mesh(np.array(devs), ("x",))
    zspec = NamedSharding(mesh, P(None, "x"))       # shard frames across cores
    rspec = NamedSharding(mesh, P())                # replicated

    # out_shardings replicated: sharded-array D2H fetch is broken under the
    # axon PJRT proxy, so all-gather the (small) output on device instead.
    fn = jax.jit(model, in_shardings=(zspec, rspec), out_shardings=rspec)

    def run_sharded(z_np, params_np):
        z = jax.device_put(z_np, zspec)
        p = jax.tree_util.tree_map(lambda a: jax.device_put(np.asarray(a), rspec),
                                   params_np)
        return np.asarray(jax.device_get(fn(z, p))).astype(np.float32)

    def run_single(z_np, params_np):
        f1 = jax.jit(model, device=devs[0])
        z = jax.device_put(z_np, devs[0])
        p = jax.tree_util.tree_map(lambda a: jax.device_put(np.asarray(a), devs[0]),
                                   params_np)
        return np.asarray(jax.device_get(f1(z, p))).astype(np.float32)

    return run_sharded, run_single


def kernel(z_codes: np.ndarray, params) -> np.ndarray:
    if "fn" not in _COMPILED:
        _COMPILED["fn"] = _build()
    run_sharded, run_single = _COMPILED["fn"]

    z_np = np.asarray(z_codes, np.float32)
    try:
        return run_sharded(z_np, params)
    except Exception as e:  # sharded compile/exec failed -> replicated fallback
        print("sharded path failed (%s); falling back to single-core" % type(e).__name__,
              flush=True)
        return run_single(z_np, params)
